# revision 38
# baseline (speedup 1.0000x reference)
"""AxileAttention Trainium2 kernel (self-contained).

Problem: x[8,64,256,256] fp32; per-channel weights *_w[64,256,256], biases *_b[64,256,256]:
    q = einsum("bchw,cwv->bchv", x, query_w) + query_b   (same for k with key_*, v with var_*)
    out = softmax(q*k, axis=-1) * v

Strategy (8 NeuronCores, SPMD via run_bass_kernel_spmd):
  * Shard the channel axis C=64 -> 8 channels/core; weights+biases sharded the same
    way, x sliced per core. Each core computes 64 (batch, channel) pairs.
  * Host pre-transposes x to xT[b,cc,w,h'] (h' in the interleaved order h = 2j+m)
    so the contraction dim w lands on SBUF partitions with 2KB-contiguous DMA runs.
  * HBM traffic trimmed from 46MB/core (all-f32) to 32.5MB/core: wv/bq/bk/bv ship
    as fp16 (they only affect the output linearly / add small absolute offsets to
    q,k), and the output is written fp16 and cast back to f32 on host. x/wq/wk
    stay fp32 (s=q*k enters exp, so s needs small absolute error while |s| can
    reach ~2e3; fp16/bf16 there fails the 2e-2 tolerance).
  * Per pair: 3 PSUM banks kk = [k_m0|k_m1] and qv_m = [q_m|v_m] (m=0,1). Biases
    are pre-loaded into PSUM via fp16 identity-matmuls (start=True) against the
    fp16 bias tiles (no host/DVE cast needed), then 8 f32r data matmuls accumulate
    on top (f32r = 1 cycle/col at N>=256). wv is DVE-cast fp16->f32r once per
    channel and rides in the same [wq|wv] moving tile as wq. kk's accumulation
    group is ordered first so its ScalarE evacuation overlaps the qv matmuls.
  * Softmax: ScalarE evacuates k (PSUM->SBUF copy, FD=512); a custom fused DVE op
    (TTR_MIN_NEG_ANT) computes s_neg = -(q*k) and the per-row -max in one pass
    (NOTE: the native tensor_tensor_reduce encodes but fails at runtime on HW —
    keep the custom op); ScalarE Exp with per-partition bias and accumulated row
    sums; DVE fast reciprocal; fused scalar_tensor_tensor computes
    out = (p * 1/sum) * v with v read straight from PSUM (no v evacuation),
    writing fp16 directly.
  * DMA queue order per channel: bk/bqv first on the ACT HWDGE queue and
    xT(b=0)/wk ahead of wqv on the gpsimd queue, so the first preload+kk matmuls
    of a channel are never DMA-starved. x stays one cast-DMA per batch (batching
    2 batches/DMA measured slower on HW SWDGE despite simming neutral).

Engine budget per core (timeline-sim, 64 pairs): PE 123.2us busy (the wall:
3072 data cols + 1536 bias-preload cols per pair at 1 col/cycle @2.4GHz),
ACT 115us (k-evac + 2 exp + 187ns/accum-read), DVE 107us (ttr/recip/stt),
DMA 93us (32.5MB), Pool 84us (SWDGE descriptor gen). Further scheduling fixes:
weight pool triple-buffered (wts bufs=3 removes ~0.27us channel-boundary PE
stalls), and a dummy exp emitted after the first channel's scalar-queue DMA
issues pre-triggers the lazy ACT exp-table load (~2.7us on HW) during the cold
fill. Putting loads on HWDGE via f32r-declared DRAM tensors sims WORSE (the
~630ns/DMA HWDGE ring cost x 176 DMAs rivals PE) -- kept on gpsimd SWDGE.
Sim total 134.8us vs baseline's 184.9us; HW measurements swing +/-20% with
tunnel load (best clean sample 146us vs baseline 203.2us).
"""
import sys

sys.path.insert(0, "/opt/trn_rl_repo")

import numpy as np

import concourse.bacc as bacc
import concourse.tile as tile
import concourse.dve_ops as dve_ops
from concourse import mybir
from concourse.masks import make_identity
from concourse.dve_spec import C0, C1, Spec, Src0, Src1, lower, minn, _has_src1
from concourse.dve_uop import DveOpSpec

F32 = mybir.dt.float32
F32R = mybir.dt.float32r
F16 = mybir.dt.float16

B = 8        # batch
C = 64       # channels total
CCH = 8      # channels per core
NCORES = 8
HP = 2       # h partition-tiles (h = 2j + m interleave)
KT = 2       # w partition-tiles (w = 2p + k interleave)
H = W = V = 256


def _make_ttr_min():
    """Custom DVE op: out = (in0*in1)*s1 ; accum_out = min(s0, row-min of out).
    Called with s1=-1, s0=+BIG: out = -(q*k), accum = -rowmax(q*k)."""
    name = "TTR_MIN_NEG_ANT"
    for op in dve_ops.OPS:
        if op.name == name:
            return op
    spec = Spec(
        body=Src0 * Src1 * C1,
        accum=minn,
        accum_init=C0,
        reference=lambda in0, in1, s0, s1, imm2: (
            np.asarray(in0, np.float32) * in1 * s1
        ),
    )
    row = dve_ops._CUSTOM_DVE_ROW_BASE + len(dve_ops.OPS)
    assert row < 0x20
    shas = {
        ver: DveOpSpec(name=name, opcode=row, uops=lower(spec, ver=ver),
                       rd1_en=_has_src1(spec)).sha(ver)
        for ver in ("v3", "v4")
    }
    op = dve_ops.DveOp(name, spec, subdim=False, uops_sha=shas)
    dve_ops.OPS.append(op)
    dve_ops.CUSTOM_DVE_SPECS[name] = spec
    dve_ops._SUB_OPCODE_FOR_NAME[name] = row
    return op


def _build_nc(reps=1, v_psum=True, out_fp16=True, recip_fast=True,
              kk_first=True, ps_bufs=2, ttr_native=False, kk_bufs=None,
              xt_bufs=4, xt_early=True, xb2=False, out_split=False, sb_bufs=3,
              out_bufs=6, hwdge_loads=False, act_warmup=True):
    ttr_min = _make_ttr_min()
    nc = bacc.Bacc("TRN2", target_bir_lowering=False, debug=False)
    # f32r is a 4-byte tag over f32 bits, so x/wq/wk can be declared f32r in
    # DRAM and loaded with plain same-dtype DMAs on the HWDGE queues (sync/
    # scalar engines) instead of gpsimd cast-DMAs -- no SWDGE descriptor-gen
    # work on the Pool engine, and the cold fill spreads across two queues.
    LD = F32R if hwdge_loads else F32
    xs = nc.dram_tensor("xs", [B, CCH, W, H], LD, kind="ExternalInput").ap()
    wq = nc.dram_tensor("wq", [CCH, W, V], LD, kind="ExternalInput").ap()
    wk = nc.dram_tensor("wk", [CCH, W, V], LD, kind="ExternalInput").ap()
    wv = nc.dram_tensor("wv", [CCH, W, V], F16, kind="ExternalInput").ap()
    bq = nc.dram_tensor("bq", [CCH, H, V], F16, kind="ExternalInput").ap()
    bk = nc.dram_tensor("bk", [CCH, H, V], F16, kind="ExternalInput").ap()
    bv = nc.dram_tensor("bv", [CCH, H, V], F16, kind="ExternalInput").ap()
    OD = F16 if out_fp16 else F32
    o = nc.dram_tensor("o", [B, CCH, H, V], OD, kind="ExternalOutput").ap()

    with tile.TileContext(nc) as tc:
        with (
            tc.tile_pool(name="const", bufs=1) as cpool,
            tc.tile_pool(name="wts", bufs=3) as wpool,
            tc.tile_pool(name="sb", bufs=sb_bufs) as sb,
            tc.tile_pool(name="ps", bufs=ps_bufs, space="PSUM") as ps,
        ):
            ident = cpool.tile([128, 128], F32)
            make_identity(nc, ident[:])
            ident_h = cpool.tile([128, 128], F16)
            nc.vector.tensor_copy(ident_h[:], ident[:])
            warmed = [False]

            def _act_warmup():
                # trigger the lazy exp ACT-table load (~2.7us) during the cold
                # DMA fill instead of stalling the first pair's softmax. Must
                # be emitted AFTER the first channel's scalar-queue dma_starts
                # (anything queued behind the table load would be delayed).
                if warmed[0] or not act_warmup:
                    return
                warmed[0] = True
                warm = cpool.tile([128, 1], F32)
                nc.scalar.activation(warm[:], ident[:, 0:1],
                                     mybir.ActivationFunctionType.Exp)

            def _body():
              B2 = 2 if xb2 else 1
              # hwdge_loads: True = x+weights on HWDGE, "w" = weights only
              ld_eng = nc.sync if hwdge_loads is True else nc.gpsimd

              def _load_xt(b, cc):
                  t = sb.tile([128, B2, KT, H], F32R, tag="xT", bufs=xt_bufs, name="xT")
                  ld_eng.dma_start(
                      t[:], xs[b:b + B2, cc].rearrange("b (p k) h -> p b k h", k=KT))
                  return t

              for cc in range(CCH):
                # x for the channel's first batch is DMA'd ahead of the weight
                # tiles so the PE pipeline fills sooner (same gpsimd queue).
                xts = {}
                if xt_early:
                    xts[0] = _load_xt(0, cc)
                # q/k weights straight into f32r via gpsimd cast-DMA; wv ships
                # fp16 and is DVE-cast into the shared [wq|wv] moving tile.
                # Rows interleaved (w=2p+k / h=2p+m) -> all DMA runs >=512B.
                # Queue order matters: the first ops of a channel need bk (kk
                # bias preload) and wk (kk data), so those DMAs go first.
                bk_mm = wpool.tile([128, HP, V], F16, tag="bk_h")
                nc.scalar.dma_start(bk_mm[:], bk[cc].rearrange("(p m) v -> p m v", m=HP))
                wk_mm = wpool.tile([128, KT, V], F32R, tag="wk_r")
                wk_eng = nc.scalar if hwdge_loads else nc.gpsimd
                wk_eng.dma_start(wk_mm[:], wk[cc].rearrange("(p k) v -> p k v", k=KT))
                wqv_mm = wpool.tile([128, KT, 512], F32R, tag="wqv_r")
                wq_eng = nc.sync if hwdge_loads else nc.gpsimd
                wq_eng.dma_start(wqv_mm[:, :, 0:V], wq[cc].rearrange("(p k) v -> p k v", k=KT))
                bqv_mm = wpool.tile([128, HP, 512], F16, tag="bqv_h")
                nc.scalar.dma_start(bqv_mm[:, :, 0:V], bq[cc].rearrange("(p m) v -> p m v", m=HP))
                nc.scalar.dma_start(bqv_mm[:, :, V:2 * V], bv[cc].rearrange("(p m) v -> p m v", m=HP))
                wv_h = wpool.tile([128, KT, V], F16, tag="wv_h")
                nc.scalar.dma_start(wv_h[:], wv[cc].rearrange("(p k) v -> p k v", k=KT))
                _act_warmup()
                nc.vector.tensor_copy(wqv_mm[:, :, V:2 * V], wv_h[:])

                xt_cur = None
                for b in range(B):
                    # xT load straight into f32r (gpsimd cast-DMA rounds)
                    if b % B2 == 0:
                        xt_cur = xts.pop(b) if b in xts else _load_xt(b, cc)
                    xT = xt_cur[:, b % B2]

                    # matmuls: bias preload (fp16 identity MM, start=True) + accumulate.
                    # kk finishes first so its ScalarE evacuation overlaps the qv matmuls.
                    qv_bank = [ps.tile([128, 512], F32, tag=f"qv{m}", name=f"qv{m}")
                               for m in range(HP)]
                    kk_bank = ps.tile([128, 512], F32, tag="kk",
                                      **({"bufs": kk_bufs} if kk_bufs else {}))
                    nc.tensor.matmul(kk_bank[:], ident_h[:],
                                     bk_mm[:].rearrange("p m v -> p (m v)"),
                                     start=True, stop=False)
                    for m in range(HP):
                        nc.tensor.matmul(qv_bank[m][:], ident_h[:], bqv_mm[:, m],
                                         start=True, stop=False)
                    if kk_first:
                        for m in range(HP):
                            for k in range(KT):
                                lq = xT[:, k, m * 128:(m + 1) * 128]
                                nc.tensor.matmul(kk_bank[:, m * 256:(m + 1) * 256], lq, wk_mm[:, k],
                                                 start=False, stop=(k == KT - 1 and m == HP - 1),
                                                 skip_group_check=True)
                        for m in range(HP):
                            for k in range(KT):
                                lq = xT[:, k, m * 128:(m + 1) * 128]
                                nc.tensor.matmul(qv_bank[m][:], lq, wqv_mm[:, k],
                                                 start=False, stop=(k == KT - 1),
                                                 skip_group_check=True)
                    else:
                        for m in range(HP):
                            for k in range(KT):
                                last = k == KT - 1
                                lq = xT[:, k, m * 128:(m + 1) * 128]
                                nc.tensor.matmul(qv_bank[m][:], lq, wqv_mm[:, k],
                                                 start=False, stop=last,
                                                 skip_group_check=True)
                                nc.tensor.matmul(kk_bank[:, m * 256:(m + 1) * 256], lq, wk_mm[:, k],
                                                 start=False, stop=(last and m == HP - 1),
                                                 skip_group_check=True)

                    # softmax chain
                    k_sb = sb.tile([128, 512], F32, tag="ksb")
                    nc.scalar.copy(k_sb[:], kk_bank[:])
                    s_sb = sb.tile([128, HP, 256], F32, tag="s")
                    mneg = sb.tile([128, HP], F32, tag="mneg")
                    for m in range(HP):
                        if ttr_native:
                            nc.vector.tensor_tensor_reduce(
                                out=s_sb[:, m],
                                in0=qv_bank[m][:, 0:256],
                                in1=k_sb[:, m * 256:(m + 1) * 256],
                                scale=-1.0, scalar=3.0e38,
                                op0=mybir.AluOpType.mult,
                                op1=mybir.AluOpType.min,
                                accum_out=mneg[:, m:m + 1],
                            )
                        else:
                            nc.vector._custom_dve(
                                ttr_min,
                                out=s_sb[:, m],
                                in0=qv_bank[m][:, 0:256],
                                in1=k_sb[:, m * 256:(m + 1) * 256],
                                s0=3.0e38, s1=-1.0,
                                accum_out=mneg[:, m:m + 1],
                            )
                    p_sb = sb.tile([128, HP, 256], F32, tag="p")
                    sums = sb.tile([128, HP], F32, tag="sums")
                    for m in range(HP):
                        nc.scalar.activation(
                            p_sb[:, m], s_sb[:, m],
                            mybir.ActivationFunctionType.Exp,
                            bias=mneg[:, m:m + 1], scale=-1.0,
                            accum_out=sums[:, m:m + 1],
                        )
                    r_sb = sb.tile([128, HP], F32, tag="r")
                    if recip_fast:
                        nc.vector.reciprocal_approx_fast(r_sb[:], sums[:])
                    else:
                        nc.vector.reciprocal(r_sb[:], sums[:])
                    out_sb = sb.tile([128, HP, 256], OD, tag="out", bufs=out_bufs)
                    o_dst = o[b, cc].rearrange("(p m) v -> p m v", m=HP)
                    for m in range(HP):
                        vsrc = (qv_bank[m][:, 256:512] if v_psum else None)
                        nc.vector.scalar_tensor_tensor(
                            out_sb[:, m], p_sb[:, m], r_sb[:, m:m + 1], vsrc,
                            op0=mybir.AluOpType.mult, op1=mybir.AluOpType.mult)
                        if out_split:
                            nc.sync.dma_start(o_dst[:, m], out_sb[:, m])
                    if not out_split:
                        nc.sync.dma_start(o_dst, out_sb[:])

            if reps > 1:
                # hardware loop: same program size, reps× the work (for timing)
                with tc.For_i(0, reps):
                    _body()
            else:
                _body()
    nc.compile()
    return nc


def _host_xT(xc):
    """[B, CC, H, W] -> xT [B, CC, W, H'] with H' enumerating h as f = m*128 + j
    <-> h = 2j + m (matches the kernel's interleaved row mapping)."""
    B_, C_, H_, W_ = xc.shape
    xt = xc.transpose(0, 1, 3, 2)
    xt = xt.reshape(B_, C_, W_, H_ // 2, 2).swapaxes(-1, -2)
    return np.ascontiguousarray(xt.reshape(B_, C_, W_, H_))


def shard_inputs(inputs):
    x = np.asarray(inputs["x"], np.float32)
    query_w, key_w, var_w = inputs["query_w"], inputs["key_w"], inputs["var_w"]
    query_b, key_b, var_b = inputs["query_b"], inputs["key_b"], inputs["var_b"]
    in_maps = []
    for c in range(NCORES):
        sl = slice(c * CCH, (c + 1) * CCH)
        in_maps.append({
            "xs": _host_xT(x[:, sl]),
            "wq": np.ascontiguousarray(np.asarray(query_w, np.float32)[sl]),
            "wk": np.ascontiguousarray(np.asarray(key_w, np.float32)[sl]),
            "wv": np.ascontiguousarray(np.asarray(var_w)[sl].astype(np.float16)),
            "bq": np.ascontiguousarray(np.asarray(query_b)[sl].astype(np.float16)),
            "bk": np.ascontiguousarray(np.asarray(key_b)[sl].astype(np.float16)),
            "bv": np.ascontiguousarray(np.asarray(var_b)[sl].astype(np.float16)),
        })
    return in_maps


def kernel(x, query_w, key_w, var_w, query_b, key_b, var_b):
    from concourse.bass_utils import run_bass_kernel_spmd

    in_maps = shard_inputs(dict(x=x, query_w=query_w, key_w=key_w, var_w=var_w,
                                query_b=query_b, key_b=key_b, var_b=var_b))
    nc = _build_nc()
    res = run_bass_kernel_spmd(nc, in_maps, list(range(NCORES)))
    out = np.empty((B, C, H, V), np.float32)
    for c in range(NCORES):
        out[:, c * CCH:(c + 1) * CCH] = res.results[c]["o"]
    return out


# revision 56
# speedup vs baseline: 1.0212x; 1.0212x over previous
"""AxileAttention Trainium2 kernel (self-contained).

Problem: x[8,64,256,256] fp32; per-channel weights *_w[64,256,256], biases *_b[64,256,256]:
    q = einsum("bchw,cwv->bchv", x, query_w) + query_b   (same for k with key_*, v with var_*)
    out = softmax(q*k, axis=-1) * v

Strategy (8 NeuronCores, SPMD via run_bass_kernel_spmd):
  * Shard the channel axis C=64 -> 8 channels/core; weights+biases sharded the same
    way, x sliced per core. Each core computes 64 (batch, channel) pairs.
  * Host pre-transposes x to xT[b,cc,w,h'] (h' in the interleaved order h = 2j+m)
    so the contraction dim w lands on SBUF partitions with 2KB-contiguous DMA runs.
  * HBM traffic trimmed from 46MB/core (all-f32) to 32.5MB/core: wv/bq/bk/bv ship
    as fp16 (they only affect the output linearly / add small absolute offsets to
    q,k), and the output is written fp16 and cast back to f32 on host. x/wq/wk
    stay fp32 (s=q*k enters exp, so s needs small absolute error while |s| can
    reach ~2e3; fp16/bf16 there fails the 2e-2 tolerance).
  * Per pair: 3 PSUM banks kk = [k_m0|k_m1] and qv_m = [q_m|v_m] (m=0,1). Biases
    are pre-loaded into PSUM via fp16 identity-matmuls (start=True) against the
    fp16 bias tiles (no host/DVE cast needed), then 8 f32r data matmuls accumulate
    on top (f32r = 1 cycle/col at N>=256). wv is DVE-cast fp16->f32r once per
    channel and rides in the same [wq|wv] moving tile as wq. kk's accumulation
    group is ordered first so its ScalarE evacuation overlaps the qv matmuls.
  * Softmax: ScalarE evacuates k (PSUM->SBUF copy, FD=512); a custom fused DVE op
    (TTR_MIN_NEG_ANT) computes s_neg = -(q*k) and the per-row -max in one pass
    (NOTE: the native tensor_tensor_reduce encodes but fails at runtime on HW —
    keep the custom op); ScalarE Exp with per-partition bias and accumulated row
    sums; DVE fast reciprocal; fused scalar_tensor_tensor computes
    out = (p * 1/sum) * v with v read straight from PSUM (no v evacuation),
    writing fp16 directly.
  * DMA queue order per channel: bk/bqv first on the ACT HWDGE queue and
    xT(b=0)/wk ahead of wqv on the gpsimd queue, so the first preload+kk matmuls
    of a channel are never DMA-starved. x stays one cast-DMA per batch (batching
    2 batches/DMA measured slower on HW SWDGE despite simming neutral).

Engine budget per core (timeline-sim, 64 pairs): PE 123.2us busy (the wall:
3072 data cols + 1536 bias-preload cols per pair at 1 col/cycle @2.4GHz),
ACT 115us (k-evac + 2 exp + 187ns/accum-read), DVE 107us (ttr/recip/stt),
DMA 93us (32.5MB), Pool 84us (SWDGE descriptor gen). Further scheduling fixes:
weight pool triple-buffered (wts bufs=3 removes ~0.27us channel-boundary PE
stalls), and a dummy exp emitted after the first channel's scalar-queue DMA
issues pre-triggers the lazy ACT exp-table load (~2.7us on HW) during the cold
fill. Putting loads on HWDGE via f32r-declared DRAM tensors sims WORSE (the
~630ns/DMA HWDGE ring cost x 176 DMAs rivals PE) -- kept on gpsimd SWDGE.
Sim total 134.8us vs baseline's 184.9us; HW measurements swing +/-20% with
tunnel load (best clean sample 146us vs baseline 203.2us).
"""
import sys

sys.path.insert(0, "/opt/trn_rl_repo")

import numpy as np

import concourse.bacc as bacc
import concourse.tile as tile
import concourse.dve_ops as dve_ops
from concourse import mybir
from concourse.masks import make_identity
from concourse.dve_spec import C0, C1, Spec, Src0, Src1, lower, minn, _has_src1
from concourse.dve_uop import DveOpSpec

F32 = mybir.dt.float32
F32R = mybir.dt.float32r
F16 = mybir.dt.float16

B = 8        # batch
C = 64       # channels total
CCH = 8      # channels per core
NCORES = 8
HP = 2       # h partition-tiles (h = 2j + m interleave)
KT = 2       # w partition-tiles (w = 2p + k interleave)
H = W = V = 256


def _make_ttr_min():
    """Custom DVE op: out = (in0*in1)*s1 ; accum_out = min(s0, row-min of out).
    Called with s1=-1, s0=+BIG: out = -(q*k), accum = -rowmax(q*k)."""
    name = "TTR_MIN_NEG_ANT"
    for op in dve_ops.OPS:
        if op.name == name:
            return op
    spec = Spec(
        body=Src0 * Src1 * C1,
        accum=minn,
        accum_init=C0,
        reference=lambda in0, in1, s0, s1, imm2: (
            np.asarray(in0, np.float32) * in1 * s1
        ),
    )
    row = dve_ops._CUSTOM_DVE_ROW_BASE + len(dve_ops.OPS)
    assert row < 0x20
    shas = {
        ver: DveOpSpec(name=name, opcode=row, uops=lower(spec, ver=ver),
                       rd1_en=_has_src1(spec)).sha(ver)
        for ver in ("v3", "v4")
    }
    op = dve_ops.DveOp(name, spec, subdim=False, uops_sha=shas)
    dve_ops.OPS.append(op)
    dve_ops.CUSTOM_DVE_SPECS[name] = spec
    dve_ops._SUB_OPCODE_FOR_NAME[name] = row
    return op


def _build_nc(reps=1, v_psum=True, out_fp16=True, recip_fast=True,
              kk_first=True, ps_bufs=2, ttr_native=False, kk_bufs=None,
              xt_bufs=4, xt_early=True, xb2=False, out_split=False, sb_bufs=3,
              out_bufs=6, hwdge_loads=False, act_warmup=True, qv_bufs=None,
              pe_warmup=0):
    ttr_min = _make_ttr_min()
    nc = bacc.Bacc("TRN2", target_bir_lowering=False, debug=False)
    # f32r is a 4-byte tag over f32 bits, so x/wq/wk can be declared f32r in
    # DRAM and loaded with plain same-dtype DMAs on the HWDGE queues (sync/
    # scalar engines) instead of gpsimd cast-DMAs -- no SWDGE descriptor-gen
    # work on the Pool engine, and the cold fill spreads across two queues.
    LD = F32R if hwdge_loads else F32
    xs = nc.dram_tensor("xs", [B, CCH, W, H], LD, kind="ExternalInput").ap()
    wq = nc.dram_tensor("wq", [CCH, W, V], LD, kind="ExternalInput").ap()
    wk = nc.dram_tensor("wk", [CCH, W, V], LD, kind="ExternalInput").ap()
    wv = nc.dram_tensor("wv", [CCH, W, V], F16, kind="ExternalInput").ap()
    bq = nc.dram_tensor("bq", [CCH, H, V], F16, kind="ExternalInput").ap()
    bk = nc.dram_tensor("bk", [CCH, H, V], F16, kind="ExternalInput").ap()
    bv = nc.dram_tensor("bv", [CCH, H, V], F16, kind="ExternalInput").ap()
    OD = F16 if out_fp16 else F32
    o = nc.dram_tensor("o", [B, CCH, H, V], OD, kind="ExternalOutput").ap()

    with tile.TileContext(nc) as tc:
        with (
            tc.tile_pool(name="const", bufs=1) as cpool,
            tc.tile_pool(name="wts", bufs=3) as wpool,
            tc.tile_pool(name="sb", bufs=sb_bufs) as sb,
            tc.tile_pool(name="ps", bufs=ps_bufs, space="PSUM") as ps,
        ):
            ident = cpool.tile([128, 128], F32)
            make_identity(nc, ident[:])
            ident_h = cpool.tile([128, 128], F16)
            nc.vector.tensor_copy(ident_h[:], ident[:])
            if pe_warmup:
                # burn the PE HAM cold-clock window during the cold DMA fill:
                # dummy matmuls on zeroed scratch (one accum group, one reader)
                warm_sb = cpool.tile([128, 256], F32R)
                nc.gpsimd.memset(warm_sb[:], 0.0)
            warmed = [False]

            def _act_warmup():
                # trigger the lazy exp ACT-table load (~2.7us) during the cold
                # DMA fill instead of stalling the first pair's softmax. Must
                # be emitted AFTER the first channel's scalar-queue dma_starts
                # (anything queued behind the table load would be delayed).
                if warmed[0] or not act_warmup:
                    return
                warmed[0] = True
                warm = cpool.tile([128, 1], F32)
                nc.scalar.activation(warm[:], ident_h[:, 0:1],
                                     mybir.ActivationFunctionType.Exp)

            def _body():
              B2 = 2 if xb2 else 1
              # hwdge_loads: True = x+weights on HWDGE, "w" = weights only
              ld_eng = nc.sync if hwdge_loads is True else nc.gpsimd

              def _load_xt(b, cc):
                  t = sb.tile([128, B2, KT, H], F32R, tag="xT", bufs=xt_bufs, name="xT")
                  ld_eng.dma_start(
                      t[:], xs[b:b + B2, cc].rearrange("b (p k) h -> p b k h", k=KT))
                  return t

              for cc in range(CCH):
                # x for the channel's first batch is DMA'd ahead of the weight
                # tiles so the PE pipeline fills sooner (same gpsimd queue).
                xts = {}
                if xt_early:
                    xts[0] = _load_xt(0, cc)
                if pe_warmup and cc == 0:
                    warm_ps = ps.tile([128, 256], F32, tag="warm", bufs=1)
                    for i in range(pe_warmup):
                        nc.tensor.matmul(warm_ps[:], warm_sb[:, 0:128], warm_sb[:],
                                         start=(i == 0), stop=(i == pe_warmup - 1),
                                         skip_group_check=True)
                    warm_rd = cpool.tile([128, 1], F32)
                    nc.vector.tensor_copy(warm_rd[:], warm_ps[:, 0:1])
                # q/k weights straight into f32r via gpsimd cast-DMA; wv ships
                # fp16 and is DVE-cast into the shared [wq|wv] moving tile.
                # Rows interleaved (w=2p+k / h=2p+m) -> all DMA runs >=512B.
                # Queue order matters: the first ops of a channel need bk (kk
                # bias preload) and wk (kk data), so those DMAs go first.
                bk_mm = wpool.tile([128, HP, V], F16, tag="bk_h")
                nc.scalar.dma_start(bk_mm[:], bk[cc].rearrange("(p m) v -> p m v", m=HP))
                wk_mm = wpool.tile([128, KT, V], F32R, tag="wk_r")
                wk_eng = nc.scalar if hwdge_loads else nc.gpsimd
                wk_eng.dma_start(wk_mm[:], wk[cc].rearrange("(p k) v -> p k v", k=KT))
                wqv_mm = wpool.tile([128, KT, 512], F32R, tag="wqv_r")
                wq_eng = nc.sync if hwdge_loads else nc.gpsimd
                wq_eng.dma_start(wqv_mm[:, :, 0:V], wq[cc].rearrange("(p k) v -> p k v", k=KT))
                bqv_mm = wpool.tile([128, HP, 512], F16, tag="bqv_h")
                nc.scalar.dma_start(bqv_mm[:, :, 0:V], bq[cc].rearrange("(p m) v -> p m v", m=HP))
                nc.scalar.dma_start(bqv_mm[:, :, V:2 * V], bv[cc].rearrange("(p m) v -> p m v", m=HP))
                wv_h = wpool.tile([128, KT, V], F16, tag="wv_h")
                nc.scalar.dma_start(wv_h[:], wv[cc].rearrange("(p k) v -> p k v", k=KT))
                _act_warmup()
                nc.vector.tensor_copy(wqv_mm[:, :, V:2 * V], wv_h[:])

                xt_cur = None
                for b in range(B):
                    # xT load straight into f32r (gpsimd cast-DMA rounds)
                    if b % B2 == 0:
                        xt_cur = xts.pop(b) if b in xts else _load_xt(b, cc)
                    xT = xt_cur[:, b % B2]

                    # matmuls: bias preload (fp16 identity MM, start=True) + accumulate.
                    # kk finishes first so its ScalarE evacuation overlaps the qv matmuls.
                    qv_bank = [ps.tile([128, 512], F32, tag=f"qv{m}", name=f"qv{m}",
                                       **({"bufs": qv_bufs} if qv_bufs else {}))
                               for m in range(HP)]
                    kk_bank = ps.tile([128, 512], F32, tag="kk",
                                      **({"bufs": kk_bufs} if kk_bufs else {}))
                    nc.tensor.matmul(kk_bank[:], ident_h[:],
                                     bk_mm[:].rearrange("p m v -> p (m v)"),
                                     start=True, stop=False)
                    for m in range(HP):
                        nc.tensor.matmul(qv_bank[m][:], ident_h[:], bqv_mm[:, m],
                                         start=True, stop=False)
                    if kk_first:
                        for m in range(HP):
                            for k in range(KT):
                                lq = xT[:, k, m * 128:(m + 1) * 128]
                                nc.tensor.matmul(kk_bank[:, m * 256:(m + 1) * 256], lq, wk_mm[:, k],
                                                 start=False, stop=(k == KT - 1 and m == HP - 1),
                                                 skip_group_check=True)
                        for m in range(HP):
                            for k in range(KT):
                                lq = xT[:, k, m * 128:(m + 1) * 128]
                                nc.tensor.matmul(qv_bank[m][:], lq, wqv_mm[:, k],
                                                 start=False, stop=(k == KT - 1),
                                                 skip_group_check=True)
                    else:
                        for m in range(HP):
                            for k in range(KT):
                                last = k == KT - 1
                                lq = xT[:, k, m * 128:(m + 1) * 128]
                                nc.tensor.matmul(qv_bank[m][:], lq, wqv_mm[:, k],
                                                 start=False, stop=last,
                                                 skip_group_check=True)
                                nc.tensor.matmul(kk_bank[:, m * 256:(m + 1) * 256], lq, wk_mm[:, k],
                                                 start=False, stop=(last and m == HP - 1),
                                                 skip_group_check=True)

                    # softmax chain
                    k_sb = sb.tile([128, 512], F32, tag="ksb")
                    nc.scalar.copy(k_sb[:], kk_bank[:])
                    s_sb = sb.tile([128, HP, 256], F32, tag="s")
                    mneg = sb.tile([128, HP], F32, tag="mneg")
                    for m in range(HP):
                        if ttr_native:
                            nc.vector.tensor_tensor_reduce(
                                out=s_sb[:, m],
                                in0=qv_bank[m][:, 0:256],
                                in1=k_sb[:, m * 256:(m + 1) * 256],
                                scale=-1.0, scalar=3.0e38,
                                op0=mybir.AluOpType.mult,
                                op1=mybir.AluOpType.min,
                                accum_out=mneg[:, m:m + 1],
                            )
                        else:
                            nc.vector._custom_dve(
                                ttr_min,
                                out=s_sb[:, m],
                                in0=qv_bank[m][:, 0:256],
                                in1=k_sb[:, m * 256:(m + 1) * 256],
                                s0=3.0e38, s1=-1.0,
                                accum_out=mneg[:, m:m + 1],
                            )
                    p_sb = sb.tile([128, HP, 256], F32, tag="p")
                    sums = sb.tile([128, HP], F32, tag="sums")
                    for m in range(HP):
                        nc.scalar.activation(
                            p_sb[:, m], s_sb[:, m],
                            mybir.ActivationFunctionType.Exp,
                            bias=mneg[:, m:m + 1], scale=-1.0,
                            accum_out=sums[:, m:m + 1],
                        )
                    r_sb = sb.tile([128, HP], F32, tag="r")
                    if recip_fast:
                        nc.vector.reciprocal_approx_fast(r_sb[:], sums[:])
                    else:
                        nc.vector.reciprocal(r_sb[:], sums[:])
                    out_sb = sb.tile([128, HP, 256], OD, tag="out", bufs=out_bufs)
                    o_dst = o[b, cc].rearrange("(p m) v -> p m v", m=HP)
                    for m in range(HP):
                        vsrc = (qv_bank[m][:, 256:512] if v_psum else None)
                        nc.vector.scalar_tensor_tensor(
                            out_sb[:, m], p_sb[:, m], r_sb[:, m:m + 1], vsrc,
                            op0=mybir.AluOpType.mult, op1=mybir.AluOpType.mult)
                        if out_split:
                            nc.sync.dma_start(o_dst[:, m], out_sb[:, m])
                    if not out_split:
                        nc.sync.dma_start(o_dst, out_sb[:])

            if reps > 1:
                # hardware loop: same program size, reps× the work (for timing)
                with tc.For_i(0, reps):
                    _body()
            else:
                _body()
    nc.compile()
    return nc


def _host_xT(xc):
    """[B, CC, H, W] -> xT [B, CC, W, H'] with H' enumerating h as f = m*128 + j
    <-> h = 2j + m (matches the kernel's interleaved row mapping)."""
    B_, C_, H_, W_ = xc.shape
    xt = xc.transpose(0, 1, 3, 2)
    xt = xt.reshape(B_, C_, W_, H_ // 2, 2).swapaxes(-1, -2)
    return np.ascontiguousarray(xt.reshape(B_, C_, W_, H_))


def shard_inputs(inputs):
    x = np.asarray(inputs["x"], np.float32)
    query_w, key_w, var_w = inputs["query_w"], inputs["key_w"], inputs["var_w"]
    query_b, key_b, var_b = inputs["query_b"], inputs["key_b"], inputs["var_b"]
    in_maps = []
    for c in range(NCORES):
        sl = slice(c * CCH, (c + 1) * CCH)
        in_maps.append({
            "xs": _host_xT(x[:, sl]),
            "wq": np.ascontiguousarray(np.asarray(query_w, np.float32)[sl]),
            "wk": np.ascontiguousarray(np.asarray(key_w, np.float32)[sl]),
            "wv": np.ascontiguousarray(np.asarray(var_w)[sl].astype(np.float16)),
            "bq": np.ascontiguousarray(np.asarray(query_b)[sl].astype(np.float16)),
            "bk": np.ascontiguousarray(np.asarray(key_b)[sl].astype(np.float16)),
            "bv": np.ascontiguousarray(np.asarray(var_b)[sl].astype(np.float16)),
        })
    return in_maps


def kernel(x, query_w, key_w, var_w, query_b, key_b, var_b):
    from concourse.bass_utils import run_bass_kernel_spmd

    in_maps = shard_inputs(dict(x=x, query_w=query_w, key_w=key_w, var_w=var_w,
                                query_b=query_b, key_b=key_b, var_b=var_b))
    nc = _build_nc()
    res = run_bass_kernel_spmd(nc, in_maps, list(range(NCORES)))
    out = np.empty((B, C, H, V), np.float32)
    for c in range(NCORES):
        out[:, c * CCH:(c + 1) * CCH] = res.results[c]["o"]
    return out
